# revision 26
# baseline (speedup 1.0000x reference)
"""Causal attention kernel for 8 TRN2 NeuronCores.

Problem: B=4, S=4096, D=1024 single-head causal attention with QKV projection.
  q/k/v = x @ W{q,k,v}.T ; out = softmax(tril(q k^T)/sqrt(D)) @ v

Sharding: core c -> batch b = c//2, parity p = c%2. Each core owns the 16 seq
blocks (128 rows) of batch b with block-index parity p ("striped" sequence
parallelism -> balanced causal work). v quarters are exchanged between the
two cores of a batch with pair-wise AllGathers issued as soon as each quarter
is projected, fully hidden under the V pass.

NO q or k projection on device at all: scores satisfy
  s^T = x^T . H   with   H = M x_own^T,  M = Wk^T Wq  (precomputed on host).
M folds both projection weights into one d x d matrix, so the whole Q pass,
its DRAM round-trip, and the per-group q reloads vanish (-55us of PE time
and -14MB of HBM traffic vs projecting q explicitly); numerically the single
bf16 M quantization measures slightly BETTER than the two-stage q/H path.

PE-roofline notes (from traces of prior versions): warm back-to-back N=512
matmuls run at ~216ns (2.4GHz streaming, LDWEIGHTS hidden); the tensor
engine must be kept on long single-PSUM-chain bursts -- cycling PSUM banks
every matmul (8 interleaved chains) re-throttles the clock to ~2.0GHz
("HAM psum-queue depth-cycling"). Other design points:
  * exact block-causal: each key block's score matmul is narrowed to the
    q-blocks it can feed (rhs slice [r_min*128:512]) and PV matmuls of
    fully-masked (kb, qb) pairs are skipped outright (they contribute 0).
    The program compiles the union geometry over the two parities; the 8
    per-group "edge" blocks get a per-core [128,128] mask (tri / zeros /
    ones) so one SPMD program serves both parities. Union work equals the
    worse parity's exact work, which set the critical path anyway.
  * softmax denominators cost no big matmuls: pt tiles accumulate on the
    (mostly idle) DVE into l_run; per q-block one tiny fp32 matmul
    l_run[:, qb].T @ ones[128,1] yields l as a [128,1] per-partition scalar,
    and 1/l folds into the ctx eviction scale.
  * PE warmup matmuls on const data ramp the clock during the initial DMAs;
    input transfers are split into small pieces interleaved across the
    sync/scalar/gpsimd queues so no matmul waits on a monolithic transfer.
"""

import sys
import types

import numpy as np

sys.path.insert(0, "/opt/trn_rl_repo")

# run_bass_kernel_spmd imports antenv.axon_hooks when BASS_TRACE is set; if
# the module is absent in this environment, install a stub that reports "no
# hook" so tracing degrades gracefully instead of crashing the run.
try:
    import antenv.axon_hooks  # noqa: F401
except ImportError:
    _hook_mod = types.ModuleType("antenv.axon_hooks")
    _hook_mod._hook = None
    _hook_mod.set_axon_ntff_profile_hook = (
        lambda h: setattr(_hook_mod, "_hook", h)
    )
    _hook_mod.get_axon_ntff_profile_hook = lambda: _hook_mod._hook
    sys.modules["antenv.axon_hooks"] = _hook_mod

import concourse.bass as bass  # noqa: E402
import concourse.mybir as mybir  # noqa: E402
import concourse.tile as tile  # noqa: E402
from concourse import bacc  # noqa: E402
from concourse.bass_utils import run_bass_kernel_spmd  # noqa: E402

import ml_dtypes  # noqa: E402

B, S, D = 4, 4096, 1024
P = 128
NB = S // P          # 32 seq blocks per batch
NLB = NB // 2        # 16 own blocks per core
SH = S // 2          # 2048 own rows per core
NG = 4               # attention q-groups of 512 rows (4 local blocks each)
SCALE = 1.0 / 32.0   # 1/sqrt(D)

BF16 = mybir.dt.bfloat16
F32 = mybir.dt.float32

_built = {}


def _group_kbs(g):
    """Union causal geometry for q-group g (q-blocks r=0..3, local blocks
    4g+r, global blocks 8g+2r+p). Key block (half, o) covers global block
    2o+half_parity; with the union over p, it is needed by q-blocks
    r >= r_min = max(0, o-4g), and for o >= 4g it is an "edge" block whose
    first valid q-block column gets the per-core mask slot `half`
    (tri for the true diagonal, zeros for the over-included block of the
    other parity, ones when fully kept)."""
    kbs = []
    for half in (0, 1):
        for o in range(4 * g + 4):
            r_min = max(0, o - 4 * g)
            kbs.append((half, o, r_min, half if o >= 4 * g else None))
    # ascending key order -> low q-blocks finish PV accumulation early
    kbs.sort(key=lambda t: 2 * t[1] + t[0])
    return kbs


def _build_nc():
    nc = bacc.Bacc("TRN2", target_bir_lowering=False, debug=False, num_devices=8)

    xtf = nc.declare_dram_parameter("xtf", [8, P, 8 * 512], BF16, isOutput=False)
    xto = nc.declare_dram_parameter("xto", [4, P, 8 * 512], BF16, isOutput=False)
    mt = nc.declare_dram_parameter("mt", [P, 8, D], BF16, isOutput=False)
    wvt = nc.declare_dram_parameter("wvt", [P, 2, 8, 512], BF16, isOutput=False)
    dmask = nc.declare_dram_parameter("dmask", [P, 2, P], BF16, isOutput=False)
    y = nc.declare_dram_parameter("y", [SH, D], BF16, isOutput=True)

    xtf3 = xtf.ap().rearrange("c p (po s) -> c p po s", po=8)   # [8, 128, 8, 512]
    xto3 = xto.ap().rearrange("c p (po s) -> c p po s", po=8)   # [4, 128, 8, 512]
    mt3 = mt.ap()
    wvt3 = wvt.ap()
    y3 = y.ap().rearrange("(nb pi) e -> nb pi e", pi=P)         # [16, 128, 1024]

    PAIRS = [[0, 1], [2, 3], [4, 5], [6, 7]]

    with tile.TileContext(nc) as tc:
        with (
            tc.tile_pool(name="dram", bufs=1, space="DRAM") as dram,
            tc.tile_pool(name="consts", bufs=1) as consts,
            tc.tile_pool(name="mp", bufs=1) as mp,
            tc.tile_pool(name="wvp", bufs=1) as wvp,
            tc.tile_pool(name="hp", bufs=2) as hp,
            tc.tile_pool(name="xqp", bufs=1) as xqp,
            tc.tile_pool(name="ktp", bufs=1) as ktp,
            tc.tile_pool(name="stg", bufs=8) as stg,
            tc.tile_pool(name="strip", bufs=32) as strip,
            tc.tile_pool(name="vload", bufs=6) as vload,
            tc.tile_pool(name="lrp", bufs=2) as lrp,
            tc.tile_pool(name="linvp", bufs=4) as linvp,
            tc.tile_pool(name="ctxs", bufs=3) as ctxs,
            tc.tile_pool(name="psum", bufs=8, space="PSUM") as psum,
        ):
            v_own = dram.tile([NLB, P, D], BF16, tag="v_own", name="v_own")
            # quarter-gather outputs: v_q[j][half*4 + o%4] = v of the
            # parity-`half` core's local block 4j + o%4
            v_q = [dram.tile([8, P, D], BF16, tag=f"v_q{j}", name=f"v_q{j}")
                   for j in range(4)]

            mask_sb = consts.tile([P, 2, P], BF16)
            ones_sb = consts.tile([P, P], BF16)
            onesf_sb = consts.tile([P, 1], F32)

            # ---- PE warmup: small const matmuls ramp the clock while the
            # first input DMAs land, so the real matmuls start warm.
            nc.gpsimd.memset(ones_sb[:], 1.0)
            nc.gpsimd.memset(onesf_sb[:], 1.0)
            warm_ps = psum.tile([P, 64], F32, tag="bank", name="warm_ps")
            for _ in range(60):
                nc.tensor.matmul(
                    warm_ps[:], lhsT=ones_sb[:], rhs=ones_sb[:, 0:64],
                    start=True, stop=True,
                )

            # x^T of own rows, SBUF-resident: feeds the V projection AND the
            # per-group H matmuls (columns g*512..(g+1)*512 are exactly
            # group g's q rows). Loaded once in fine interleaved pieces.
            xq_sb = xqp.tile([P, 4, 8, 512], BF16, name="xq_sb")
            # x^T of all rows (keys), SBUF-resident, parity order,
            # chunk-major so each load piece is per-partition contiguous
            # (1KB descriptors run ~25GB/s vs ~90GB/s at 4KB)
            xt_sb = ktp.tile([P, 8, 8, 512], BF16)

            wv0_sb = wvp.tile([P, 8, 512], BF16, tag="wv0", name="wv0_sb")
            wv1_sb = wvp.tile([P, 8, 512], BF16, tag="wv1", name="wv1_sb")
            # The first V chain consumes ALL of wv0 within ~1.7us and eh1
            # needs all of wv1 ~9us in: each half rides a different queue
            # (serializing both on gpsimd cost an 8.5us PE gap at the eh1
            # boundary of chunk 0).
            nc.gpsimd.dma_start(wv0_sb[:, 0:4], wvt3[:, 0, 0:4])
            nc.sync.dma_start(wv0_sb[:, 4:8], wvt3[:, 0, 4:8])
            nc.scalar.dma_start(wv1_sb[:, 0:4], wvt3[:, 1, 0:4])
            nc.gpsimd.dma_start(wv1_sb[:, 4:8], wvt3[:, 1, 4:8])
            nc.sync.dma_start(xq_sb[:, 0, 0:4], xto3[0][:, 0:4])
            nc.scalar.dma_start(xq_sb[:, 0, 4:8], xto3[0][:, 4:8])
            nc.scalar.dma_start(mask_sb[:], dmask.ap())
            # remaining xq chunks: c1/c3 split sync+scalar, c2 on gpsimd
            nc.sync.dma_start(xq_sb[:, 1, 0:4], xto3[1][:, 0:4])
            nc.scalar.dma_start(xq_sb[:, 1, 4:8], xto3[1][:, 4:8])
            nc.gpsimd.dma_start(xq_sb[:, 2], xto3[2])
            nc.sync.dma_start(xq_sb[:, 3, 0:4], xto3[3][:, 0:4])
            nc.scalar.dma_start(xq_sb[:, 3, 4:8], xto3[3][:, 4:8])
            m_sb = mp.tile([P, 8, D], BF16, name="m_sb")

            # ---- V pass (v = x Wv^T, own rows) -> v_own, quarter-gathered
            # as each chunk completes. Single accumulation chain per PSUM
            # tile (bank cycling would re-throttle the PE clock).
            for c in range(4):
                for eh in range(2):
                    for sb in range(4):
                        ps = psum.tile([P, 512], F32, tag="bank", name="ps_v")
                        for dc in range(8):
                            nc.tensor.matmul(
                                ps[:],
                                lhsT=xq_sb[:, c, dc, sb * P:(sb + 1) * P],
                                rhs=(wv0_sb[:, dc, :] if eh == 0
                                     else wv1_sb[:, dc, :]),
                                start=(dc == 0),
                                stop=(dc == 7),
                            )
                        vho = stg.tile([P, 512], BF16, tag="stg512", name="vho")
                        dst = v_own[c * 4 + sb][:, eh * 512:(eh + 1) * 512]
                        if sb % 2 == 0:
                            nc.vector.tensor_copy(out=vho[:], in_=ps[:])
                            nc.sync.dma_start(dst, vho[:])
                        else:
                            nc.scalar.copy(vho[:], ps[:])
                            nc.scalar.dma_start(dst, vho[:])
                nc.gpsimd.collective_compute(
                    "AllGather",
                    mybir.AluOpType.bypass,
                    replica_groups=PAIRS,
                    ins=[v_own[4 * c:4 * c + 4].opt()],
                    outs=[v_q[c][:].opt()],
                )

            # M = Wk^T Wq and the key pieces load after the V-pass inputs
            # (first needed ~50us later at H(0)/scores(0)); deferring them
            # keeps gpsimd's early bandwidth on the V-pass weights.
            nc.gpsimd.dma_start(m_sb[:], mt3)
            for c in (0, 4):
                nc.gpsimd.dma_start(xt_sb[:, c], xtf3[c])
            # remaining key pieces, needed-first order, split sync/scalar
            for c in (1, 5, 2, 6, 3, 7):
                nc.sync.dma_start(xt_sb[:, c, 0:4], xtf3[c][:, 0:4])
                nc.scalar.dma_start(xt_sb[:, c, 4:8], xtf3[c][:, 4:8])

            # ---- Attention ----
            def pass1(g):
                """Scores + exp + mask + DVE denominator accumulation for
                group g:  s^T = x^T . H,  H = M x_own^T (no q anywhere)."""
                kbs = _group_kbs(g)


                # H[b, qi] = sum_a M[a, b] x[qi, a], evicted bf16 to SBUF
                h_sb = hp.tile([P, 8, 512], BF16, tag="h", name=f"h_{g}")
                for db in range(8):
                    hps = psum.tile([P, 512], F32, tag="bank", name=f"hps_{g}_{db}")
                    for ac in range(8):
                        nc.tensor.matmul(
                            hps[:],
                            lhsT=m_sb[:, ac, db * P:(db + 1) * P],
                            rhs=xq_sb[:, g, ac, :],
                            start=(ac == 0),
                            stop=(ac == 7),
                        )
                    nc.vector.tensor_copy(out=h_sb[:, db, :], in_=hps[:])

                # running softmax-denominator accumulator (f32, on the DVE)
                l_run = lrp.tile([P, 512], F32, tag="lrun", name=f"lrun_{g}")

                pts = []
                for kb_idx, (half, o, r_min, mslot) in enumerate(kbs):
                    kc, ko = half * 4 + o // 4, (o % 4) * P
                    q0 = r_min * P
                    st_ps = psum.tile([P, 512], F32, tag="bank", name=f"st_ps_{g}")
                    for dc in range(8):
                        nc.tensor.matmul(
                            st_ps[:, q0:512],
                            lhsT=xt_sb[:, kc, dc, ko:ko + P],
                            rhs=h_sb[:, dc, q0:512],
                            start=(dc == 0),
                            stop=(dc == 7),
                        )
                    pt = strip.tile([P, 512], BF16, tag="pt", name=f"pt_{g}")
                    nc.scalar.activation(
                        pt[:, q0:512], st_ps[:, q0:512],
                        mybir.ActivationFunctionType.Exp, scale=SCALE,
                    )
                    if mslot is not None:
                        nc.vector.tensor_mul(
                            out=pt[:, q0:q0 + P], in0=pt[:, q0:q0 + P],
                            in1=mask_sb[:, mslot, :],
                        )
                    if kb_idx == 0:
                        nc.vector.tensor_copy(out=l_run[:], in_=pt[:])
                    else:
                        nc.vector.tensor_add(
                            out=l_run[:, q0:512], in0=l_run[:, q0:512],
                            in1=pt[:, q0:512],
                        )
                    pts.append(pt)
                return kbs, pts, l_run

            def pv(g, state):
                kbs, pts, l_run = state
                # denominator -> [128,1] per-partition scalars via 4 tiny
                # column matmuls (l_run slice stationary, ones moving). The
                # lp tile is allocated BEFORE the ctx tiles: the psum ring
                # then has ctx[3,1] reuse lp's slot (cheap WAR on the early
                # reciprocal) instead of lp reusing a ctx slot, which would
                # deadlock (ctx eviction needs linv <- lp).
                lp = psum.tile([P, 4], F32, tag="bank", name=f"lp_{g}")
                for qb in range(4):
                    # fp32 matmul is 4x slower per row, irrelevant at N=1.
                    # One accumulation group: start=True zeroes the whole
                    # 2KB pending-zero region, so only the first column may
                    # set it (each later column then writes its own fresh
                    # pending-zero bytes).
                    nc.tensor.matmul(
                        lp[:, qb:qb + 1],
                        lhsT=l_run[:, qb * P:(qb + 1) * P],
                        rhs=onesf_sb[:],
                        start=(qb == 0), stop=(qb == 3),
                        skip_group_check=True,
                    )
                linv = linvp.tile([P, 4], F32, tag="linv", name=f"linv_{g}")
                nc.vector.reciprocal(linv[:], lp[:])

                # last kb index needing each qb -> evict that qb's ctx early
                last_kb = {r: max(i for i, kb in enumerate(kbs) if kb[2] <= r)
                           for r in range(4)}
                ctx_ps = {
                    (qb, eh): psum.tile([P, 512], F32, tag="bank",
                                        name=f"ctx_{g}_{qb}_{eh}")
                    for qb in range(4) for eh in range(2)
                }
                started = set()

                def evict(qb):
                    for eh in range(2):
                        cs = ctxs.tile([P, 512], BF16, tag="cs", name=f"cs_{g}")
                        # normalize during eviction; alternate engines so PSUM
                        # banks free ~2x faster
                        if (qb + eh) % 2 == 0:
                            nc.scalar.mul(cs[:], ctx_ps[(qb, eh)][:],
                                          linv[:, qb:qb + 1])
                        else:
                            nc.vector.tensor_scalar_mul(
                                cs[:], ctx_ps[(qb, eh)][:], linv[:, qb:qb + 1])
                        ydst = y3[4 * g + qb, :, eh * 512:(eh + 1) * 512]
                        # whole-tile DMAs (splitting tiles lengthens the
                        # drain: descriptor-gen dominates); the last group's
                        # bunched evictions alternate queues so descriptor
                        # generation runs on two sequencers in parallel
                        if g == NG - 1 and eh == 1:
                            nc.scalar.dma_start(ydst, cs[:])
                        else:
                            nc.sync.dma_start(ydst, cs[:])

                for kb_idx, (half, o, r_min, mslot) in enumerate(kbs):
                    vsrc = v_q[o // 4]
                    vb = half * 4 + (o % 4)
                    vt = vload.tile([P, D], BF16, tag="vt", name=f"vt_{g}")
                    # Alternate queues: the narrowed late kbs consume only
                    # 2-4 matmuls (<1us) per 256KB vt load, so one queue
                    # can't keep up (measured ~2-3us stalls at each group's
                    # tail). Safe on sync by timing: all gathers complete
                    # ~100us before the first attention vt load, so the
                    # gather-semaphore wait never head-of-line blocks y
                    # evictions queued behind it.
                    if kb_idx % 2 == 0:
                        nc.gpsimd.dma_start(vt[:], vsrc[vb])
                    else:
                        nc.sync.dma_start(vt[:], vsrc[vb])
                    for qb in range(r_min, 4):
                        for eh in range(2):
                            nc.tensor.matmul(
                                ctx_ps[(qb, eh)][:],
                                lhsT=pts[kb_idx][:, qb * P:(qb + 1) * P],
                                rhs=vt[:, eh * 512:(eh + 1) * 512],
                                start=((qb, eh) not in started),
                                stop=(kb_idx == last_kb[qb]),
                            )
                            started.add((qb, eh))
                    for qb in range(4):
                        if last_kb[qb] == kb_idx:
                            evict(qb)

            for g in range(NG):
                pv(g, pass1(g))

    nc.compile()
    return nc


def _host_inputs(x, Wq, Wk, Wv):
    """Build per-core input maps. x: [B,S,D] f32; W*: [D,D] f32."""
    bf = ml_dtypes.bfloat16

    # M = Wq^T Wk folds both score projections: s = x M x^T. Stored like a
    # natural [a, b] matrix, pi-major over the contraction dim a.
    M = (Wq.T @ Wk).astype(bf)
    mtx = np.ascontiguousarray(M.reshape(8, P, D).transpose(1, 0, 2))

    def w_pim(W):
        # [pi, eh, po, e'] with element = W[eh*512+e', po*128+pi]
        return np.ascontiguousarray(
            W.T.astype(bf).reshape(8, P, 2, 512).transpose(1, 2, 0, 3)
        )

    wvt = w_pim(Wv)

    # per-parity edge-block masks [128 k, 2 halves, 128 q] (block-local):
    # tri = keep iff key row <= q row. For parity 0 the half-0 edge block is
    # the true diagonal (tri) and the half-1 edge block belongs to the
    # future (zeros); for parity 1 half-0 is fully past (ones), half-1 is
    # the diagonal (tri).
    tri = (np.arange(P)[:, None] <= np.arange(P)[None, :]).astype(np.float32)
    dmask_p = []
    for p in (0, 1):
        m = np.empty((P, 2, P), np.float32)
        m[:, 0, :] = tri if p == 0 else 1.0
        m[:, 1, :] = 0.0 if p == 0 else tri
        dmask_p.append(m.astype(bf))

    in_maps = []
    xb_cache = {}
    for c in range(8):
        b, p = c // 2, c % 2
        if b not in xb_cache:
            # parity order: [even blocks | odd blocks]
            perm = [2 * j for j in range(NLB)] + [2 * j + 1 for j in range(NLB)]
            xbf = x[b].reshape(NB, P, D)[perm].reshape(S, D)
            xb_cache[b] = xbf.T.astype(bf)  # [D, S]
        xt_full = xb_cache[b]
        # [c, pi, po*512]: per-partition-contiguous chunks
        xtf_c = np.ascontiguousarray(
            xt_full.reshape(8, P, 8, 512).transpose(2, 1, 0, 3)
        ).reshape(8, P, 8 * 512)
        xto_half = xt_full[:, p * SH:(p + 1) * SH]
        xto_c = np.ascontiguousarray(
            xto_half.reshape(8, P, 4, 512).transpose(2, 1, 0, 3)
        ).reshape(4, P, 8 * 512)

        in_maps.append({
            "xtf": xtf_c,
            "xto": xto_c,
            "mt": mtx,
            "wvt": wvt,
            "dmask": dmask_p[p],
        })
    return in_maps


def kernel(**inputs):
    x = np.asarray(inputs["inputs"], np.float32)
    Wq = np.asarray(inputs["Wq"], np.float32)
    Wk = np.asarray(inputs["Wk"], np.float32)
    Wv = np.asarray(inputs["Wv"], np.float32)

    if "nc" not in _built:
        _built["nc"] = _build_nc()
    nc = _built["nc"]

    in_maps = _host_inputs(x, Wq, Wk, Wv)
    res = run_bass_kernel_spmd(nc, in_maps, core_ids=list(range(8)))

    out = np.empty((B, S, D), np.float32)
    for c in range(8):
        b, p = c // 2, c % 2
        yc = np.asarray(res.results[c]["y"], np.float32).reshape(NLB, P, D)
        ob = out[b].reshape(NB, P, D)
        for j in range(NLB):
            ob[2 * j + p] = yc[j]
    return out
